# revision 2
# baseline (speedup 1.0000x reference)
"""Chamfer-style loss kernel for Trainium2 (8 NeuronCores, SPMD).

Problem: y_pred [8192,2], y_true [8192,2] (fp32).
  d[n,m] = ||p_n - t_m||;  loss = (sum_n min_m d + sum_m min_n d) / 8192

Strategy per core k (shard y_pred rows, 1024 per core):
  - Augmented K=4 matmul on PE computes the full squared-distance tile
    S[n,m] = |p_n|^2 + |t_m|^2 - 2 p.t  directly in PSUM:
       lhsT = [-2px; -2py; 1; |p|^2]  (4 x 1024)
       rhs  = [tx; ty; |t|^2; 1]      (4 x 8192)
  - ACT copies PSUM->SBUF as bf16 (min chains run at DVE 2x mode in bf16).
  - DVE tensor_tensor(min) chains: row-mins (over m) and per-partition
    column candidates; PE transpose + DVE reduce for partition-axis col-min.
  - sqrt commutes with min, so sqrt only on the 16K final mins.
  - One AllReduce(min) over [129,64] fp32: rows 0..127 = col-min partials
    (all 8192 m), row 128 = one-hot gather of per-core row-min sqrt-sums.
  - Every core then computes the identical final scalar; host takes core 0.
"""

import sys

if "/opt/trn_rl_repo" not in sys.path:
    sys.path.insert(0, "/opt/trn_rl_repo")

import numpy as np

import concourse.bass as bass
import concourse.bacc as bacc
import concourse.tile as tile
from concourse import masks, mybir
from concourse.bass_utils import run_bass_kernel_spmd

F32 = mybir.dt.float32
BF16 = mybir.dt.bfloat16
MIN = mybir.AluOpType.min
ADD = mybir.AluOpType.add
X = mybir.AxisListType.X

N_CORES = 8
N = 8192          # y_pred points
M = 8192          # y_true points
N_LOC = N // N_CORES   # 1024 rows per core
N_BLK = N_LOC // 128   # 8 row blocks of 128 partitions
N_CHUNK = 16           # m-chunks of 512
CHUNK = M // N_CHUNK   # 512
BIG = 3.0e38

TRACE = False          # set True by test harness to capture HW profile
LAST_RESULTS = None    # BassKernelResults of the most recent run

_CACHE = {}


def _build_program():
    nc = bacc.Bacc(
        "TRN2",
        target_bir_lowering=False,
        debug=False,
        num_devices=N_CORES,
    )

    lhs_d = nc.dram_tensor("lhs", [4, N_LOC], F32, kind="ExternalInput")
    rhs_d = nc.dram_tensor("rhs", [4, M], F32, kind="ExternalInput")
    sel_d = nc.dram_tensor("sel", [1, 64], F32, kind="ExternalInput")
    selbig_d = nc.dram_tensor("selbig", [1, 64], F32, kind="ExternalInput")
    out_d = nc.dram_tensor("out", [1, 1], F32, kind="ExternalOutput")

    with tile.TileContext(nc) as tc:
        with (
            tc.tile_pool(name="const", bufs=1) as const_pool,
            tc.tile_pool(name="acc", bufs=1) as acc_pool,
            tc.tile_pool(name="chunk", bufs=3) as chunk_pool,
            tc.tile_pool(name="tree", bufs=2) as tree_pool,
            tc.tile_pool(name="fin", bufs=1) as fin_pool,
            tc.tile_pool(name="mm", bufs=2, space="PSUM") as mm_pool,
            tc.tile_pool(name="tp", bufs=2, space="PSUM") as tp_pool,
            tc.tile_pool(name="dram", bufs=1, space="DRAM") as dram_pool,
        ):
            # ---- constants / inputs to SBUF ----
            lhs_sb = const_pool.tile([4, N_LOC], F32)
            rhs_sb = const_pool.tile([4, M], F32)
            sel_sb = const_pool.tile([1, 64], F32)
            selbig_sb = const_pool.tile([1, 64], F32)
            ident = const_pool.tile([128, 128], BF16)
            ones_sb = const_pool.tile([128, 1], F32)

            nc.sync.dma_start(lhs_sb[:, :], lhs_d.ap())
            nc.sync.dma_start(rhs_sb[:, :], rhs_d.ap())
            nc.sync.dma_start(sel_sb[:, :], sel_d.ap())
            nc.sync.dma_start(selbig_sb[:, :], selbig_d.ap())
            masks.make_identity(nc, ident[:, :])
            nc.vector.memset(ones_sb[:, :], 1.0)

            # ---- persistent accumulators ----
            # row-min candidates, ping-pong buffers: [128, 8 (row-block), 512]
            rowacc_a = acc_pool.tile([128, N_BLK * CHUNK], BF16)
            rowacc_b = acc_pool.tile([128, N_BLK * CHUNK], BF16)
            # col-min per m: m = j*128 + q at [q, j]
            colminT = acc_pool.tile([128, 64], F32)
            nc.vector.memset(rowacc_a[:, :], BIG)

            rowaccs = [rowacc_a, rowacc_b]

            # ---- main loop over m-chunks ----
            for c in range(N_CHUNK):
                chunk_sb = chunk_pool.tile(
                    [128, N_BLK * CHUNK], BF16, name="chunk_sb", tag="chunk"
                )
                # PE: S tiles for all 8 row-blocks of this m-chunk.
                # PSUM groups of 3/3/2 banks so two groups pipeline in 8 banks.
                for g, (i0, ng) in enumerate([(0, 3), (3, 3), (6, 2)]):
                    mm_ps = mm_pool.tile(
                        [128, 3 * CHUNK], F32, name="mm_ps", tag="mm"
                    )
                    for j in range(ng):
                        i = i0 + j
                        nc.tensor.matmul(
                            mm_ps[:, j * CHUNK:(j + 1) * CHUNK],
                            lhs_sb[:, i * 128:(i + 1) * 128],
                            rhs_sb[:, c * CHUNK:(c + 1) * CHUNK],
                            start=True,
                            stop=True,
                        )
                    # ACT: PSUM fp32 -> SBUF bf16
                    nc.scalar.copy(
                        chunk_sb[:, i0 * CHUNK:(i0 + ng) * CHUNK],
                        mm_ps[:, 0:ng * CHUNK],
                    )

                # DVE row chain: rowacc = min(rowacc, chunk)
                src = rowaccs[c % 2]
                dst = rowaccs[(c + 1) % 2]
                nc.vector.tensor_tensor(dst[:, :], src[:, :], chunk_sb[:, :], MIN)

                # DVE col tree: min over the 8 row-blocks -> [128, 512]
                tmp1 = tree_pool.tile([128, 2048], BF16, name="tmp1", tag="t1")
                tmp2 = tree_pool.tile([128, 1024], BF16, name="tmp2", tag="t2")
                ccnd = tree_pool.tile([128, 512], BF16, name="ccnd", tag="t3")
                nc.vector.tensor_tensor(
                    tmp1[:, :], chunk_sb[:, 0:2048], chunk_sb[:, 2048:4096], MIN
                )
                nc.vector.tensor_tensor(
                    tmp2[:, :], tmp1[:, 0:1024], tmp1[:, 1024:2048], MIN
                )
                nc.vector.tensor_tensor(
                    ccnd[:, :], tmp2[:, 0:512], tmp2[:, 512:1024], MIN
                )

                # PE transpose + DVE reduce: min over the 128 partitions
                tp_ps = tp_pool.tile([128, 512], BF16, name="tp_ps", tag="tp")
                for k in range(4):
                    nc.tensor.transpose(
                        tp_ps[:, k * 128:(k + 1) * 128],
                        ccnd[:, k * 128:(k + 1) * 128],
                        ident[:, :],
                    )
                nc.vector.tensor_reduce(
                    colminT[:, c * 4:(c + 1) * 4],
                    tp_ps.rearrange("p (k q) -> p k q", k=4),
                    axis=X,
                    op=MIN,
                )

            # ---- local row-min finalization ----
            rowacc = rowaccs[N_CHUNK % 2]
            rowmin8 = fin_pool.tile([128, N_BLK], F32)
            nc.vector.tensor_reduce(
                rowmin8[:, :],
                rowacc.rearrange("p (i f) -> p i f", i=N_BLK),
                axis=X,
                op=MIN,
            )
            # clamp negatives (cancellation noise), sqrt, sum over free dim
            nc.vector.tensor_scalar_max(rowmin8[:, :], rowmin8[:, :], 0.0)
            rowd = fin_pool.tile([128, N_BLK], F32)
            rowpart = fin_pool.tile([128, 1], F32)
            nc.scalar.activation(
                rowd[:, :], rowmin8[:, :],
                mybir.ActivationFunctionType.Sqrt,
                accum_out=rowpart[:, :],
            )
            # partition sum -> scalar
            ps_row = tp_pool.tile([128, 512], F32, name="ps_row", tag="tp")
            nc.tensor.matmul(
                ps_row[0:1, 0:1], ones_sb[:, :], rowpart[:, :],
                start=True, stop=True,
            )
            rowsum_sb = fin_pool.tile([1, 1], F32)
            nc.scalar.copy(rowsum_sb[:, :], ps_row[0:1, 0:1])

            # gather slots: sel*rowsum + (1-sel)*BIG
            gat1 = fin_pool.tile([1, 64], F32)
            gat2 = fin_pool.tile([1, 64], F32)
            nc.vector.tensor_scalar_mul(gat1[:, :], sel_sb[:, :], rowsum_sb[:, 0:1])
            nc.vector.tensor_tensor(gat2[:, :], gat1[:, :], selbig_sb[:, :], ADD)

            # ---- AllReduce(min) over [129, 64] ----
            ar_in = dram_pool.tile([129, 64], F32)
            ar_out = dram_pool.tile([129, 64], F32, addr_space="Shared")
            nc.sync.dma_start(ar_in[0:128, :], colminT[:, :])
            nc.sync.dma_start(ar_in[128:129, :], gat2[:, :])
            nc.gpsimd.collective_compute(
                "AllReduce",
                MIN,
                replica_groups=[list(range(N_CORES))],
                ins=[ar_in[:, :].opt()],
                outs=[ar_out[:, :].opt()],
            )

            # ---- global finalization (identical on every core) ----
            cmin = fin_pool.tile([128, 64], F32)
            rsums = fin_pool.tile([1, 64], F32)
            nc.sync.dma_start(cmin[:, :], ar_out[0:128, :])
            nc.sync.dma_start(rsums[:, :], ar_out[128:129, :])

            nc.vector.tensor_scalar_max(cmin[:, :], cmin[:, :], 0.0)
            cd = fin_pool.tile([128, 64], F32)
            colpart = fin_pool.tile([128, 1], F32)
            nc.scalar.activation(
                cd[:, :], cmin[:, :],
                mybir.ActivationFunctionType.Sqrt,
                accum_out=colpart[:, :],
            )
            ps_col = tp_pool.tile([128, 512], F32, name="ps_col", tag="tp")
            nc.tensor.matmul(
                ps_col[0:1, 0:1], ones_sb[:, :], colpart[:, :],
                start=True, stop=True,
            )
            colsum_sb = fin_pool.tile([1, 1], F32)
            nc.scalar.copy(colsum_sb[:, :], ps_col[0:1, 0:1])

            rtot = fin_pool.tile([1, 1], F32)
            nc.vector.tensor_reduce(rtot[:, :], rsums[0:1, 0:8], axis=X, op=ADD)

            fin = fin_pool.tile([1, 1], F32)
            nc.vector.tensor_tensor(fin[:, :], colsum_sb[:, :], rtot[:, :], ADD)
            out_sb = fin_pool.tile([1, 1], F32)
            nc.scalar.mul(out_sb[:, :], fin[:, :], 1.0 / M)
            nc.sync.dma_start(out_d.ap(), out_sb[:, :])

    nc.compile()
    return nc


def _prep_inputs(y_pred, y_true):
    p = np.ascontiguousarray(np.asarray(y_pred, dtype=np.float32).reshape(-1, 2))
    t = np.ascontiguousarray(np.asarray(y_true, dtype=np.float32).reshape(-1, 2))
    assert p.shape == (N, 2) and t.shape == (M, 2)

    rhs = np.empty((4, M), dtype=np.float32)
    rhs[0] = t[:, 0]
    rhs[1] = t[:, 1]
    rhs[2] = t[:, 0] * t[:, 0] + t[:, 1] * t[:, 1]
    rhs[3] = 1.0

    in_maps = []
    for k in range(N_CORES):
        pk = p[k * N_LOC:(k + 1) * N_LOC]
        lhs = np.empty((4, N_LOC), dtype=np.float32)
        lhs[0] = -2.0 * pk[:, 0]
        lhs[1] = -2.0 * pk[:, 1]
        lhs[2] = 1.0
        lhs[3] = pk[:, 0] * pk[:, 0] + pk[:, 1] * pk[:, 1]
        sel = np.zeros((1, 64), dtype=np.float32)
        sel[0, k] = 1.0
        selbig = np.full((1, 64), BIG, dtype=np.float32)
        selbig[0, k] = 0.0
        in_maps.append({"lhs": lhs, "rhs": rhs, "sel": sel, "selbig": selbig})
    return in_maps


def kernel(y_pred, y_true):
    global LAST_RESULTS
    if "nc" not in _CACHE:
        _CACHE["nc"] = _build_program()
    nc = _CACHE["nc"]
    in_maps = _prep_inputs(y_pred, y_true)
    res = run_bass_kernel_spmd(
        nc,
        in_maps,
        core_ids=list(range(N_CORES)),
        trace=TRACE,
    )
    LAST_RESULTS = res
    return np.asarray(res.results[0]["out"], dtype=np.float32).reshape(())[()]


# revision 3
# speedup vs baseline: 1.2026x; 1.2026x over previous
"""Chamfer-style loss kernel for Trainium2 (8 NeuronCores, SPMD).

Problem: y_pred [8192,2], y_true [8192,2] (fp32).
  d[n,m] = ||p_n - t_m||;  loss = (sum_n min_m d + sum_m min_n d) / 8192

Strategy per core k (shard y_pred rows, 1024 per core):
  - Augmented K=4 matmul on PE computes the full squared-distance tile
    S[n,m] = |p_n|^2 + |t_m|^2 - 2 p.t  directly in PSUM:
       lhsT = [-2px; -2py; 1; |p|^2]  (4 x 1024)
       rhs  = [tx; ty; |t|^2; 1]      (4 x 8192)
    K=4 uses only 4 PE rows, so 4 matmuls are packed onto row quadrants
    via tile_position (lhs/rhs replicated at partition offsets 0/32/64/96).
  - ACT copies PSUM->SBUF as bf16 (min chains run at DVE 2x mode in bf16).
  - DVE tensor_tensor(min) chains: row-mins (over m) and per-partition
    column candidates; DMA xbar transpose + DVE reduce for the
    partition-axis col-min.
  - sqrt commutes with min, so sqrt only on the 16K final mins.
  - One AllReduce(min) over [129,64] fp32: rows 0..127 = col-min partials
    (all 8192 m), row 128 = one-hot gather of per-core row-min sqrt-sums.
  - Every core then computes the identical final scalar; host takes core 0.
"""

import sys

if "/opt/trn_rl_repo" not in sys.path:
    sys.path.insert(0, "/opt/trn_rl_repo")

import numpy as np

import concourse.bass as bass
import concourse.bacc as bacc
import concourse.tile as tile
from concourse import mybir
from concourse.bass_utils import run_bass_kernel_spmd

F32 = mybir.dt.float32
BF16 = mybir.dt.bfloat16
MIN = mybir.AluOpType.min
ADD = mybir.AluOpType.add
X = mybir.AxisListType.X

N_CORES = 8
N = 8192          # y_pred points
M = 8192          # y_true points
N_LOC = N // N_CORES   # 1024 rows per core
N_BLK = N_LOC // 128   # 8 row blocks of 128 partitions
N_CHUNK = 16           # m-chunks of 512
CHUNK = M // N_CHUNK   # 512
BIG = 3.0e38

TRACE = False          # set True by test harness to capture HW profile
LAST_RESULTS = None    # BassKernelResults of the most recent run

_CACHE = {}


def _build_program():
    nc = bacc.Bacc(
        "TRN2",
        target_bir_lowering=False,
        debug=False,
        num_devices=N_CORES,
    )

    lhs_d = nc.dram_tensor("lhs", [4, N_LOC], F32, kind="ExternalInput")
    rhs_d = nc.dram_tensor("rhs", [4, M], F32, kind="ExternalInput")
    sel_d = nc.dram_tensor("sel", [1, 64], F32, kind="ExternalInput")
    selbig_d = nc.dram_tensor("selbig", [1, 64], F32, kind="ExternalInput")
    out_d = nc.dram_tensor("out", [1, 1], F32, kind="ExternalOutput")

    with tile.TileContext(nc) as tc:
        with (
            tc.tile_pool(name="const", bufs=1) as const_pool,
            tc.tile_pool(name="acc", bufs=1) as acc_pool,
            tc.tile_pool(name="chunk", bufs=3) as chunk_pool,
            tc.tile_pool(name="tree", bufs=2) as tree_pool,
            tc.tile_pool(name="fin", bufs=1) as fin_pool,
            tc.tile_pool(name="mm", bufs=2, space="PSUM") as mm_pool,
            tc.tile_pool(name="dram", bufs=1, space="DRAM") as dram_pool,
        ):
            # ---- constants / inputs to SBUF ----
            # lhs/rhs replicated at partition offsets 0/32/64/96 so four
            # K=4 matmuls can run concurrently on the four PE row quadrants.
            lhs_sb = const_pool.tile([128, N_LOC], F32, padded_shape=[128, N_LOC])
            rhs_sb = const_pool.tile([128, M], F32, padded_shape=[128, M])
            sel_sb = const_pool.tile([1, 64], F32)
            selbig_sb = const_pool.tile([1, 64], F32)
            ones_sb = const_pool.tile([128, 1], F32)

            nc.sync.dma_start(lhs_sb[0:4, :], lhs_d.ap())
            nc.sync.dma_start(rhs_sb[0:4, :], rhs_d.ap())
            for r in range(1, 4):
                nc.sync.dma_start(lhs_sb[32 * r:32 * r + 4, :], lhs_sb[0:4, :])
                nc.sync.dma_start(rhs_sb[32 * r:32 * r + 4, :], rhs_sb[0:4, :])
            nc.sync.dma_start(sel_sb[:, :], sel_d.ap())
            nc.sync.dma_start(selbig_sb[:, :], selbig_d.ap())
            nc.vector.memset(ones_sb[:, :], 1.0)

            # ---- persistent accumulators ----
            # row-min candidates, ping-pong buffers: [128, 8 (row-block), 512]
            rowacc_a = acc_pool.tile([128, N_BLK * CHUNK], BF16)
            rowacc_b = acc_pool.tile([128, N_BLK * CHUNK], BF16)
            # col-min per m: m = j*128 + q at [q, j]
            colminT = acc_pool.tile([128, 64], F32)
            nc.vector.memset(rowacc_a[:, :], BIG)

            rowaccs = [rowacc_a, rowacc_b]

            # ---- main loop over m-chunks ----
            for c in range(N_CHUNK):
                chunk_sb = chunk_pool.tile(
                    [128, N_BLK * CHUNK], BF16, name="chunk_sb", tag="chunk"
                )
                # PE: S tiles for all 8 row-blocks of this m-chunk,
                # packed 4-at-a-time onto the PE row quadrants.
                for g in range(2):
                    mm_ps = mm_pool.tile(
                        [128, 4 * CHUNK], F32, name="mm_ps", tag="mm"
                    )
                    for r in range(4):
                        i = 4 * g + r
                        nc.tensor.matmul(
                            mm_ps[:, r * CHUNK:(r + 1) * CHUNK],
                            lhs_sb[32 * r:32 * r + 4, i * 128:(i + 1) * 128],
                            rhs_sb[32 * r:32 * r + 4, c * CHUNK:(c + 1) * CHUNK],
                            start=True,
                            stop=True,
                            tile_position=(32 * r, 0),
                        )
                    # ACT: PSUM fp32 -> SBUF bf16
                    nc.scalar.copy(
                        chunk_sb[:, g * 4 * CHUNK:(g + 1) * 4 * CHUNK],
                        mm_ps[:, :],
                    )

                # DVE row chain: rowacc = min(rowacc, chunk)
                src = rowaccs[c % 2]
                dst = rowaccs[(c + 1) % 2]
                nc.vector.tensor_tensor(dst[:, :], src[:, :], chunk_sb[:, :], MIN)

                # DVE col tree: min over the 8 row-blocks -> [128, 512]
                tmp1 = tree_pool.tile([128, 2048], BF16, name="tmp1", tag="t1")
                tmp2 = tree_pool.tile([128, 1024], BF16, name="tmp2", tag="t2")
                ccnd = tree_pool.tile([128, 512], BF16, name="ccnd", tag="t3")
                nc.vector.tensor_tensor(
                    tmp1[:, :], chunk_sb[:, 0:2048], chunk_sb[:, 2048:4096], MIN
                )
                nc.vector.tensor_tensor(
                    tmp2[:, :], tmp1[:, 0:1024], tmp1[:, 1024:2048], MIN
                )
                nc.vector.tensor_tensor(
                    ccnd[:, :], tmp2[:, 0:512], tmp2[:, 512:1024], MIN
                )

                # DMA xbar transpose + DVE reduce: min over the 128 partitions
                tpT = tree_pool.tile([128, 512], BF16, name="tpT", tag="tpT")
                for k in range(4):
                    nc.sync.dma_start_transpose(
                        tpT[:, k * 128:(k + 1) * 128],
                        ccnd[:, k * 128:(k + 1) * 128],
                    )
                nc.vector.tensor_reduce(
                    colminT[:, c * 4:(c + 1) * 4],
                    tpT.rearrange("p (k q) -> p k q", k=4),
                    axis=X,
                    op=MIN,
                )

            # ---- local row-min finalization ----
            rowacc = rowaccs[N_CHUNK % 2]
            rowmin8 = fin_pool.tile([128, N_BLK], F32)
            nc.vector.tensor_reduce(
                rowmin8[:, :],
                rowacc.rearrange("p (i f) -> p i f", i=N_BLK),
                axis=X,
                op=MIN,
            )
            # clamp negatives (cancellation noise), sqrt, sum over free dim
            nc.vector.tensor_scalar_max(rowmin8[:, :], rowmin8[:, :], 0.0)
            rowd = fin_pool.tile([128, N_BLK], F32)
            rowpart = fin_pool.tile([128, 1], F32)
            nc.scalar.activation(
                rowd[:, :], rowmin8[:, :],
                mybir.ActivationFunctionType.Sqrt,
                accum_out=rowpart[:, :],
            )
            # partition sum -> scalar
            ps_row = mm_pool.tile([128, 4 * CHUNK], F32, name="ps_row", tag="mm")
            nc.tensor.matmul(
                ps_row[0:1, 0:1], ones_sb[:, :], rowpart[:, :],
                start=True, stop=True,
            )
            rowsum_sb = fin_pool.tile([1, 1], F32)
            nc.scalar.copy(rowsum_sb[:, :], ps_row[0:1, 0:1])

            # gather slots: sel*rowsum + (1-sel)*BIG
            gat1 = fin_pool.tile([1, 64], F32)
            gat2 = fin_pool.tile([1, 64], F32)
            nc.vector.tensor_scalar_mul(gat1[:, :], sel_sb[:, :], rowsum_sb[:, 0:1])
            nc.vector.tensor_tensor(gat2[:, :], gat1[:, :], selbig_sb[:, :], ADD)

            # ---- AllReduce(min) over [129, 64] ----
            ar_in = dram_pool.tile([129, 64], F32)
            ar_out = dram_pool.tile([129, 64], F32, addr_space="Shared")
            nc.sync.dma_start(ar_in[0:128, :], colminT[:, :])
            nc.sync.dma_start(ar_in[128:129, :], gat2[:, :])
            nc.gpsimd.collective_compute(
                "AllReduce",
                MIN,
                replica_groups=[list(range(N_CORES))],
                ins=[ar_in[:, :].opt()],
                outs=[ar_out[:, :].opt()],
            )

            # ---- global finalization (identical on every core) ----
            cmin = fin_pool.tile([128, 64], F32)
            rsums = fin_pool.tile([1, 64], F32)
            nc.sync.dma_start(cmin[:, :], ar_out[0:128, :])
            nc.sync.dma_start(rsums[:, :], ar_out[128:129, :])

            nc.vector.tensor_scalar_max(cmin[:, :], cmin[:, :], 0.0)
            cd = fin_pool.tile([128, 64], F32)
            colpart = fin_pool.tile([128, 1], F32)
            nc.scalar.activation(
                cd[:, :], cmin[:, :],
                mybir.ActivationFunctionType.Sqrt,
                accum_out=colpart[:, :],
            )
            ps_col = mm_pool.tile([128, 4 * CHUNK], F32, name="ps_col", tag="mm")
            nc.tensor.matmul(
                ps_col[0:1, 0:1], ones_sb[:, :], colpart[:, :],
                start=True, stop=True,
            )
            colsum_sb = fin_pool.tile([1, 1], F32)
            nc.scalar.copy(colsum_sb[:, :], ps_col[0:1, 0:1])

            rtot = fin_pool.tile([1, 1], F32)
            nc.vector.tensor_reduce(rtot[:, :], rsums[0:1, 0:8], axis=X, op=ADD)

            fin = fin_pool.tile([1, 1], F32)
            nc.vector.tensor_tensor(fin[:, :], colsum_sb[:, :], rtot[:, :], ADD)
            out_sb = fin_pool.tile([1, 1], F32)
            nc.scalar.mul(out_sb[:, :], fin[:, :], 1.0 / M)
            nc.sync.dma_start(out_d.ap(), out_sb[:, :])

    nc.compile()
    return nc


def _prep_inputs(y_pred, y_true):
    p = np.ascontiguousarray(np.asarray(y_pred, dtype=np.float32).reshape(-1, 2))
    t = np.ascontiguousarray(np.asarray(y_true, dtype=np.float32).reshape(-1, 2))
    assert p.shape == (N, 2) and t.shape == (M, 2)

    rhs = np.empty((4, M), dtype=np.float32)
    rhs[0] = t[:, 0]
    rhs[1] = t[:, 1]
    rhs[2] = t[:, 0] * t[:, 0] + t[:, 1] * t[:, 1]
    rhs[3] = 1.0

    in_maps = []
    for k in range(N_CORES):
        pk = p[k * N_LOC:(k + 1) * N_LOC]
        lhs = np.empty((4, N_LOC), dtype=np.float32)
        lhs[0] = -2.0 * pk[:, 0]
        lhs[1] = -2.0 * pk[:, 1]
        lhs[2] = 1.0
        lhs[3] = pk[:, 0] * pk[:, 0] + pk[:, 1] * pk[:, 1]
        sel = np.zeros((1, 64), dtype=np.float32)
        sel[0, k] = 1.0
        selbig = np.full((1, 64), BIG, dtype=np.float32)
        selbig[0, k] = 0.0
        in_maps.append({"lhs": lhs, "rhs": rhs, "sel": sel, "selbig": selbig})
    return in_maps


def kernel(y_pred, y_true):
    global LAST_RESULTS
    if "nc" not in _CACHE:
        _CACHE["nc"] = _build_program()
    nc = _CACHE["nc"]
    in_maps = _prep_inputs(y_pred, y_true)
    res = run_bass_kernel_spmd(
        nc,
        in_maps,
        core_ids=list(range(N_CORES)),
        trace=TRACE,
    )
    LAST_RESULTS = res
    return np.asarray(res.results[0]["out"], dtype=np.float32).reshape(())[()]
